# revision 16
# baseline (speedup 1.0000x reference)
"""Trainium2 Bass kernel for a padded-attention transformer encoder layer.

Shapes (hardcoded): src [4, 2048, 1024], 16 heads x 64, d_ff 4096, 8 cores.

Sharding: each core computes the full layer for 1024 output tokens
(batch = core//2, token half = core%2). Inputs are host-transposed
(feature-major) and host-rolled so every core's query tokens are columns
0:1024 of its srcT; attention over keys is permutation invariant so the
roll only permutes the contraction order.

On-core dataflow (everything feature-on-partitions, fp32r matmuls):
  xT = rmsnorm_T(srcT)                    (partition-dim reduce via ones-matmul)
  qT2/kT2 per head pair from xT; v (token-major) via xT-stationary matmuls
  scoresT[t,s] = kT.T @ qT; expT = exp(0.125*scoresT)  (no max-sub needed)
  v_aug = [v*kmask | kmask]  -> attnV matmul yields o and softmax denom at once
  o normalized by qmask/denom, out-proj accumulated into saT += Wo.T-part
  yT = rmsnorm_T(saT); h1 = silu(yT@W1)*(yT@V1) stored bf16 over dead xT space
  outT = saT + h1 @ W2  (bf16 matmul, fp32 accum)
"""

import sys

sys.path.insert(0, "/opt/trn_rl_repo")

import numpy as np
import ml_dtypes

import concourse.bass as bass
import concourse.mybir as mybir
import concourse.tile as tile
from concourse import bacc
from concourse.bass_utils import run_bass_kernel_spmd

F32 = mybir.dt.float32
F32R = mybir.dt.float32r
BF16 = mybir.dt.bfloat16
EXP = mybir.ActivationFunctionType.Exp
SILU = mybir.ActivationFunctionType.Silu
SQRT = mybir.ActivationFunctionType.Sqrt

B, S, D, H, DK, DFF = 4, 2048, 1024, 16, 64, 4096
SQ = 1024          # query tokens per core
DC = D // 128      # 8 d-chunks
TC = S // 128      # 16 token chunks
FC = DFF // 128    # 32 dff chunks
NPAIR = H // 2     # 8 head pairs
SCALE = DK ** -0.5


def r32(ap):
    return ap.bitcast(F32R)


def build():
    nc = bacc.Bacc("TRN2", target_bir_lowering=False, debug=False, num_devices=8)

    srcT = nc.dram_tensor("srcT", [D, S], F32, kind="ExternalInput").ap()
    kmask = nc.dram_tensor("kmask", [S, 1], F32, kind="ExternalInput").ap()
    qmaskd = nc.dram_tensor("qmask", [1, SQ], F32, kind="ExternalInput").ap()
    wq = nc.dram_tensor("wq", [D, D], F32, kind="ExternalInput").ap()
    wk = nc.dram_tensor("wk", [D, D], F32, kind="ExternalInput").ap()
    wv = nc.dram_tensor("wv", [D, D], F32, kind="ExternalInput").ap()
    wo = nc.dram_tensor("wo", [D, D], F32, kind="ExternalInput").ap()
    w1 = nc.dram_tensor("w1", [D, DFF], F32, kind="ExternalInput").ap()
    v1 = nc.dram_tensor("v1", [D, DFF], F32, kind="ExternalInput").ap()
    w2b = nc.dram_tensor("w2b", [DFF, D], BF16, kind="ExternalInput").ap()
    outT = nc.dram_tensor("outT", [D, SQ], F32, kind="ExternalOutput").ap()

    # persistent SBUF arrays. h1 (bf16, FFN intermediate) aliases xt's bytes:
    # xt is fully consumed before the first h1 write (enforced by the tracked
    # byte-range deps through the saT chain), and the verifier needs separate
    # memory locations for the fp32r- and bf16-consumed data.
    xt, h1t = [], []
    for i in range(DC):
        xt.append(nc.alloc_sbuf_tensor(f"xt{i}", [128, S], F32R).ap())
        off = nc.sbuf_base - S * 4
        h1t.append(nc.alloc_sbuf_tensor_at(f"h1t{i}", [128, 2 * S], BF16, offset=off).ap())
    # sat holds the attention residual stream saT, then is scaled in place to
    # yT = rmsnorm_T(saT); the final residual is reconstructed as yT * rms.
    sat = [nc.alloc_sbuf_tensor(f"sat{i}", [128, SQ], F32R).ap() for i in range(DC)]
    # v for one quarter-round (2 pairs = 4 heads), augmented with kmask col
    vq = [nc.alloc_sbuf_tensor(f"vq{i}", [128, 4 * 65], F32R).ap() for i in range(TC)]

    with nc.allow_low_precision(reason="fp32r matmul operand rounding; fp32 PSUM accumulation"), \
         tile.TileContext(nc) as tc:
        with (
            tc.tile_pool(name="kt2p", bufs=2) as kt2p,
            tc.tile_pool(name="qt2p", bufs=2) as qt2p,
            tc.tile_pool(name="expp", bufs=3) as expp,
            tc.tile_pool(name="tmp", bufs=3) as tmp,
            tc.tile_pool(name="wst", bufs=8) as wst,
            tc.tile_pool(name="w2st", bufs=8) as w2st,
            tc.tile_pool(name="wost", bufs=4) as wost,
            tc.tile_pool(name="consts", bufs=1) as consts,
            tc.tile_pool(name="sm", bufs=1) as sm,
            tc.tile_pool(name="psA", bufs=2, space="PSUM") as psA,
            tc.tile_pool(name="psB", bufs=2, space="PSUM") as psB,
        ):
            # ---- constants ----
            onesf = consts.tile([128, 128], F32, tag="onesf")
            nc.vector.memset(onesf[:], 1.0)
            ones4 = consts.tile([128, 4], F32R, tag="ones4")
            nc.vector.tensor_copy(ones4[:], onesf[:, 0:4])
            ones1 = consts.tile([1, 128], F32R, tag="ones1")
            nc.vector.tensor_copy(ones1[:], onesf[0:1, :])
            km = consts.tile([128, TC], F32, tag="km")
            for ti in range(TC):
                nc.sync.dma_start(out=km[:, ti : ti + 1], in_=kmask[ti * 128 : (ti + 1) * 128, :])
            qm = consts.tile([1, SQ], F32, tag="qm")
            nc.sync.dma_start(out=qm[:], in_=qmaskd[:])

            # ---- P0: xT = rmsnorm_T(srcT), in two 1024-col halves ----
            for th in range(2):
                hs = slice(th * 1024, (th + 1) * 1024)
                ss = psB.tile([4, 1024], F32, tag="acc")
                for dc in range(DC):
                    ld = tmp.tile([128, 1024], F32R, tag="tmp")
                    nc.sync.dma_start(out=ld[:], in_=r32(srcT[dc * 128 : (dc + 1) * 128, hs]))
                    sq = tmp.tile([128, 1024], F32R, tag="tmp")
                    nc.vector.tensor_mul(sq[:], ld[:], ld[:])
                    for n2 in range(2):
                        ns = slice(n2 * 512, (n2 + 1) * 512)
                        nc.tensor.matmul(ss[:, ns], ones4[:], sq[:, ns],
                                         start=(dc == 0), stop=(dc == DC - 1))
                rms = sm.tile([1, 1024], F32, tag="rms")
                nc.scalar.activation(rms[:], ss[0:1, :], SQRT, scale=1.0 / D)
                inv = sm.tile([1, 1024], F32R, tag="inv")
                nc.vector.reciprocal(inv[:], rms[:])
                invB = psA.tile([128, 1024], F32, tag="mm")
                for n2 in range(2):
                    ns = slice(n2 * 512, (n2 + 1) * 512)
                    nc.tensor.matmul(invB[:, ns], ones1[:], inv[:, ns], start=True, stop=True)
                for dc in range(DC):
                    ld2 = tmp.tile([128, 1024], F32R, tag="tmp")
                    nc.sync.dma_start(out=ld2[:], in_=r32(srcT[dc * 128 : (dc + 1) * 128, hs]))
                    nc.vector.tensor_mul(xt[dc][:, hs], ld2[:], invB[:])

            # ---- P1: attention ----
            for p in range(NPAIR):
                vr, lp = p // 2, p % 2
                if lp == 0:
                    # v for pairs {2vr, 2vr+1}: heads 4vr..4vr+3 (cols 256 of wv)
                    wvts = []
                    for dc in range(DC):
                        wvt = wst.tile([128, 256], F32R, tag="wst")
                        nc.sync.dma_start(
                            out=wvt[:],
                            in_=r32(wv[dc * 128 : (dc + 1) * 128, vr * 256 : (vr + 1) * 256]),
                        )
                        wvts.append(wvt)
                    for ti in range(TC):
                        vps = psA.tile([128, 1024], F32, tag="mm")
                        for dc in range(DC):
                            nc.tensor.matmul(
                                vps[:, 0:256],
                                xt[dc][:, ti * 128 : (ti + 1) * 128],
                                wvts[dc][:],
                                start=(dc == 0), stop=(dc == DC - 1),
                            )
                        # vq[ti] = [v*km | km] interleaved per head: [128, 4, 65]
                        dst = vq[ti].rearrange("p (h c) -> p h c", c=65)
                        src3 = vps[:, 0:256].rearrange("p (h c) -> p h c", c=64)
                        nc.vector.tensor_scalar_mul(dst[:, :, 0:64], src3, km[:, ti : ti + 1])
                        for l in range(4):
                            nc.vector.tensor_copy(dst[:, l, 64:65], r32(km[:, ti : ti + 1]))

                # kT2 / qT2 for this pair
                kps = [psA.tile([128, 1024], F32, tag="mm", name=f"kps{half}")
                       for half in range(2)]
                qps = psB.tile([128, 1024], F32, tag="acc")
                for dc in range(DC):
                    wkt = wst.tile([128, 128], F32R, tag="wst")
                    nc.sync.dma_start(out=wkt[:], in_=r32(wk[dc * 128 : (dc + 1) * 128, p * 128 : (p + 1) * 128]))
                    wqt = wst.tile([128, 128], F32R, tag="wst")
                    nc.sync.dma_start(out=wqt[:], in_=r32(wq[dc * 128 : (dc + 1) * 128, p * 128 : (p + 1) * 128]))
                    for half in range(2):
                        for n2 in range(2):
                            ns = slice(n2 * 512, (n2 + 1) * 512)
                            nc.tensor.matmul(
                                kps[half][:, ns], wkt[:],
                                xt[dc][:, half * 1024 + n2 * 512 : half * 1024 + (n2 + 1) * 512],
                                start=(dc == 0), stop=(dc == DC - 1),
                            )
                    for n2 in range(2):
                        ns = slice(n2 * 512, (n2 + 1) * 512)
                        nc.tensor.matmul(qps[:, ns], wqt[:], xt[dc][:, ns],
                                         start=(dc == 0), stop=(dc == DC - 1))
                kt2 = kt2p.tile([128, S], F32R, tag="kt2")
                for half in range(2):
                    nc.vector.tensor_copy(kt2[:, half * 1024 : (half + 1) * 1024], kps[half][:])
                qt2 = qt2p.tile([128, SQ], F32R, tag="qt2")
                nc.vector.tensor_copy(qt2[:], qps[:])

                otn = tmp.tile([128, 1024], F32R, tag="tmp")
                for h in range(2):
                    l = 2 * lp + h
                    oT = psB.tile([128, 1024], F32, tag="acc")
                    for ti in range(TC):
                        sc = psA.tile([128, 1024], F32, tag="mm")
                        for n2 in range(2):
                            ns = slice(n2 * 512, (n2 + 1) * 512)
                            nc.tensor.matmul(
                                sc[:, ns],
                                kt2[h * 64 : (h + 1) * 64, ti * 128 : (ti + 1) * 128],
                                qt2[h * 64 : (h + 1) * 64, ns],
                                start=True, stop=True,
                            )
                        ex = expp.tile([128, 1024], F32R, tag="exp")
                        nc.scalar.activation(ex[:], sc[:], EXP, scale=SCALE)
                        for n2 in range(2):
                            ns = slice(n2 * 512, (n2 + 1) * 512)
                            nc.tensor.matmul(
                                oT[0:65, ns],
                                vq[ti][:, l * 65 : (l + 1) * 65],
                                ex[:, ns],
                                start=(ti == 0), stop=(ti == TC - 1),
                            )
                    # normalize rows: o * qmask/denom
                    rec = sm.tile([1, 1024], F32, tag="rec")
                    nc.vector.reciprocal(rec[:], oT[64:65, :])
                    recq = sm.tile([1, 1024], F32R, tag="recq")
                    nc.vector.tensor_mul(recq[:], rec[:], qm[:])
                    rB = psA.tile([128, 1024], F32, tag="mm")
                    for n2 in range(2):
                        ns = slice(n2 * 512, (n2 + 1) * 512)
                        nc.tensor.matmul(rB[0:64, ns], ones1[:, 0:64], recq[:, ns],
                                         start=True, stop=True)
                    rBs = tmp.tile([128, 1024], F32, tag="tmp")
                    nc.vector.tensor_copy(rBs[0:64, :], rB[0:64, :])
                    nc.vector.tensor_mul(otn[h * 64 : (h + 1) * 64, :], oT[0:64, :], rBs[0:64, :])

                # out-proj partial: saT[dc2] (+)= wo[p-rows, dc2-cols].T @ otn
                for dc2 in range(DC):
                    wot = wost.tile([128, 128], F32R, tag="wost")
                    nc.sync.dma_start(out=wot[:], in_=r32(wo[p * 128 : (p + 1) * 128, dc2 * 128 : (dc2 + 1) * 128]))
                    pp = psA.tile([128, 1024], F32, tag="mm")
                    for n2 in range(2):
                        ns = slice(n2 * 512, (n2 + 1) * 512)
                        nc.tensor.matmul(pp[:, ns], wot[:], otn[:, ns], start=True, stop=True)
                    if p == 0:
                        srcq = tmp.tile([128, 1024], F32, tag="tmp")
                        nc.sync.dma_start(out=srcq[:], in_=srcT[dc2 * 128 : (dc2 + 1) * 128, 0:SQ])
                        nc.vector.tensor_add(sat[dc2][:], pp[:], srcq[:])
                    else:
                        nc.vector.tensor_add(sat[dc2][:], sat[dc2][:], pp[:])

            # ---- P2: saT -> yT in place; keep rmsy for the final residual ----
            ssy = psB.tile([4, 1024], F32, tag="acc")
            for dc in range(DC):
                sqy = tmp.tile([128, 1024], F32R, tag="tmp")
                nc.vector.tensor_mul(sqy[:], sat[dc][:], sat[dc][:])
                for n2 in range(2):
                    ns = slice(n2 * 512, (n2 + 1) * 512)
                    nc.tensor.matmul(ssy[:, ns], ones4[:], sqy[:, ns],
                                     start=(dc == 0), stop=(dc == DC - 1))
            rmsy = consts.tile([1, 1024], F32R, tag="rmsy")
            nc.scalar.activation(rmsy[:], ssy[0:1, :], SQRT, scale=1.0 / D)
            invy = sm.tile([1, 1024], F32R, tag="inv")
            nc.vector.reciprocal(invy[:], rmsy[:])
            invyB = psA.tile([128, 1024], F32, tag="mm")
            for n2 in range(2):
                ns = slice(n2 * 512, (n2 + 1) * 512)
                nc.tensor.matmul(invyB[:, ns], ones1[:], invy[:, ns], start=True, stop=True)
            for dc in range(DC):
                nc.vector.tensor_mul(sat[dc][:], sat[dc][:], invyB[:])

            # ---- P3: h1 = silu(yT@W1)*(yT@V1) -> bf16, overlaid on xt space ----
            h1c = []
            for fc in range(FC):
                h1c.append(h1t[fc // 4][:, (fc % 4) * 1024 : (fc % 4 + 1) * 1024])
            for fc in range(FC):
                h1w = psA.tile([128, 1024], F32, tag="mm")
                h1v = psB.tile([128, 1024], F32, tag="acc")
                for dc in range(DC):
                    w1t = wst.tile([128, 128], F32R, tag="wst")
                    nc.sync.dma_start(out=w1t[:], in_=r32(w1[dc * 128 : (dc + 1) * 128, fc * 128 : (fc + 1) * 128]))
                    v1t = wst.tile([128, 128], F32R, tag="wst")
                    nc.sync.dma_start(out=v1t[:], in_=r32(v1[dc * 128 : (dc + 1) * 128, fc * 128 : (fc + 1) * 128]))
                    for n2 in range(2):
                        ns = slice(n2 * 512, (n2 + 1) * 512)
                        nc.tensor.matmul(h1w[:, ns], w1t[:], sat[dc][:, ns],
                                         start=(dc == 0), stop=(dc == DC - 1))
                        nc.tensor.matmul(h1v[:, ns], v1t[:], sat[dc][:, ns],
                                         start=(dc == 0), stop=(dc == DC - 1))
                sil = tmp.tile([128, 1024], F32, tag="tmp")
                nc.scalar.activation(sil[:], h1w[:], SILU)
                nc.vector.tensor_mul(h1c[fc], sil[:], h1v[:])

            # ---- P4: outT = yT*rmsy + h1 @ W2   (saT reconstructed) ----
            rmsyB = psB.tile([128, 1024], F32, tag="acc")
            for n2 in range(2):
                ns = slice(n2 * 512, (n2 + 1) * 512)
                nc.tensor.matmul(rmsyB[:, ns], ones1[:], rmsy[:, ns], start=True, stop=True)
            for dc2 in range(DC):
                h2 = psA.tile([128, 1024], F32, tag="mm")
                for fc in range(FC):
                    w2t = w2st.tile([128, 128], BF16, tag="w2st")
                    nc.sync.dma_start(out=w2t[:], in_=w2b[fc * 128 : (fc + 1) * 128, dc2 * 128 : (dc2 + 1) * 128])
                    for n2 in range(2):
                        ns = slice(n2 * 512, (n2 + 1) * 512)
                        nc.tensor.matmul(h2[:, ns], w2t[:], h1c[fc][:, ns],
                                         start=(fc == 0), stop=(fc == FC - 1))
                sa_rec = tmp.tile([128, 1024], F32, tag="tmp")
                nc.vector.tensor_mul(sa_rec[:], sat[dc2][:], rmsyB[:])
                ot = tmp.tile([128, 1024], F32, tag="tmp")
                nc.vector.tensor_add(ot[:], sa_rec[:], h2[:])
                nc.sync.dma_start(out=outT[dc2 * 128 : (dc2 + 1) * 128, :], in_=ot[:])

    nc.compile()
    return nc


_NC = None


def _get_nc():
    global _NC
    if _NC is None:
        _NC = build()
    return _NC


def _build_in_maps(inputs):
    return _prep(**inputs)


def kernel(**inputs):
    in_maps = _prep(**inputs)
    res = run_bass_kernel_spmd(_get_nc(), in_maps, list(range(8)))
    out = np.empty((B, S, D), np.float32)
    for c in range(8):
        b, qh = c // 2, c % 2
        out[b, qh * SQ : (qh + 1) * SQ, :] = res.results[c]["outT"].T
    return out


def _prep(src, src_padding_mask, Wq, Wk, Wv, Wo, g1, g2, W1, V1, W2, **_):
    src = np.asarray(src, np.float32)
    valid = (~np.asarray(src_padding_mask, bool)).astype(np.float32)
    g1 = np.asarray(g1, np.float32)
    g2 = np.asarray(g2, np.float32)
    wq_cat = (np.transpose(np.asarray(Wq, np.float32), (1, 0, 2)).reshape(D, D)
              * g1[:, None]).astype(np.float32)
    wk_cat = (np.transpose(np.asarray(Wk, np.float32), (1, 0, 2)).reshape(D, D)
              * g1[:, None]).astype(np.float32)
    wv_cat = (np.transpose(np.asarray(Wv, np.float32), (1, 0, 2)).reshape(D, D)
              * g1[:, None]).astype(np.float32)
    wo_a = np.ascontiguousarray(np.asarray(Wo, np.float32))
    w1_s = np.ascontiguousarray(np.asarray(W1, np.float32) * g2[:, None])
    v1_s = np.ascontiguousarray(np.asarray(V1, np.float32) * g2[:, None])
    w2_b = np.asarray(W2, np.float32).astype(ml_dtypes.bfloat16)

    in_maps = []
    for c in range(8):
        b, qh = c // 2, c % 2
        roll = qh * SQ
        src_r = np.roll(src[b], -roll, axis=0)          # [S, D]
        srcT_c = np.ascontiguousarray(src_r.T)          # [D, S]
        km_c = np.ascontiguousarray(np.roll(valid[b], -roll)).reshape(S, 1)
        qm_c = np.ascontiguousarray(km_c[0:SQ].reshape(1, SQ))
        in_maps.append({
            "srcT": srcT_c, "kmask": km_c, "qmask": qm_c,
            "wq": wq_cat, "wk": wk_cat, "wv": wv_cat, "wo": wo_a,
            "w1": w1_s, "v1": v1_s, "w2b": w2_b,
        })
    return in_maps
